# revision 10
# baseline (speedup 1.0000x reference)
"""Trainium2 Bass kernel for nn_KuramotoCoupling.

Reference computation (B=2, S=2048, D=1024, H=16, HD=64):
    mag   = sqrt(z_re^2 + z_im^2)
    q,k,v = mag @ W{q,k,v}.T  (per-head, HD=64)
    w     = softmax(q k^T / 8)
    value_mix    = (w @ v) @ Wo.T
    phase_update = c * (cos_ph * (w @ sin_ph) - sin_ph * (w @ cos_ph))
where sin_ph = z_im / mag, cos_ph = z_re / mag  (since ph = atan2(z_im, z_re)).

Sharding: 8 cores = 2 batches x 4 head-groups (4 heads each).  Each core
gets its batch's z (full D, column-permuted so its OWN 256 channels come
first), its heads' slices of Wq/Wk/Wv (rows permuted to match, bf16), and
its j-slice of Wo^T (bf16).  Outputs: phase slice [S, 256] (concatenated
on host) and a value_mix partial [S, D] (summed over head-groups on host).

Device-side structure per core:
  A) stream z by 128-row chunks: ssq = re^2+im^2 (gpsimd square + ACT
     square + DVE add); own channels = ssq[:, 0:256] -> rinv -> sin/cos
     into X tiles [s, (head: v|sin|cos|one)] (bf16); PE-transpose ssq and
     evict through ACT Sqrt -> magT [d, s] (bf16).
  B) qT/kT = WqT.T @ magT  [ch, s] (bf16), v = magT.T @ WvT -> X v cols.
  C) flat software pipeline over 8 units (2 q-tiles x 4 heads): for unit
     N+1 the scoresT = kT.T @ qT -> ACT exp (bf16 wT, no max-sub:
     |scores/8| < ~6) interleaved at k-granularity with unit N's mixing
     out[q, v|sin|cos|one] += wT.T @ X (accumulated over all 16 k in
     PSUM).  The one-column gives the softmax denominator.  Per-u DVE
     epilogue: recip, value rows scaled, phase = (Ksin*rc)*cos -
     (Kcos*rc)*sin.  PE and ACT overlap across units.
  D) per q-tile: PE-transpose value rows (bf16) -> vT, value_mix
     partial [q, i] = vT.T @ WoT, DMA'd straight from PSUM.
"""
import sys

if "/opt/trn_rl_repo" not in sys.path:
    sys.path.insert(0, "/opt/trn_rl_repo")

import numpy as np
import ml_dtypes
from contextlib import ExitStack

import concourse.bacc as bacc
import concourse.mybir as mybir
import concourse.tile as tile
from concourse.bass_utils import run_bass_kernel_spmd
from concourse.masks import make_identity

F32 = mybir.dt.float32
F32R = mybir.dt.float32r
BF16 = mybir.dt.bfloat16
AF = mybir.ActivationFunctionType
ALU = mybir.AluOpType

B, S, D, H = 2, 2048, 1024, 16
HD = D // H
N_CORES = 8
HG = 4                   # head-groups (tensor-parallel dim)
HPC = H // HG            # heads per core = 4
CH = HPC * HD            # own channels per core = 256
P = 128
SC = S // P              # 16 s-chunks
DC = D // P              # 8 d-chunks
XW = HPC * 256           # X tile width: per head [v64|sin64|cos64|one..]
XN = 193                 # mixing moving width: v64|sin64|cos64|one


def build_kernel(sc=SC):
    """Build the SPMD program for one core.  sc = number of 128-row
    s-chunks (16 for the real problem; smaller for debug builds)."""
    s = sc * P
    kc = sc                      # k-chunks == s-chunks
    uq = max(1, sc // 2)         # q-chunks per q-tile
    n_qt = 2 if sc >= 2 else 1
    qw = uq * P
    kpu = kc // uq               # score k-chunks issued per u-step (2)
    halves = ((0, sc // 2), (sc // 2, sc)) if sc >= 4 else ((0, sc),)

    nc = bacc.Bacc("TRN2", target_bir_lowering=False, debug=False,
                   num_devices=N_CORES)

    z_re = nc.dram_tensor("z_re", [s, D], F32, kind="ExternalInput").ap()
    z_im = nc.dram_tensor("z_im", [s, D], F32, kind="ExternalInput").ap()
    wq_t = nc.dram_tensor("wq_t", [D, CH], BF16, kind="ExternalInput").ap()
    wk_t = nc.dram_tensor("wk_t", [D, CH], BF16, kind="ExternalInput").ap()
    wv_t = nc.dram_tensor("wv_t", [D, CH], BF16, kind="ExternalInput").ap()
    wo_t = nc.dram_tensor("wo_t", [CH, D], BF16, kind="ExternalInput").ap()
    coup = nc.dram_tensor("coup", [P, 1], F32, kind="ExternalInput").ap()
    phase_out = nc.dram_tensor("phase_out", [s, CH], F32,
                               kind="ExternalOutput").ap()
    vmix_out = nc.dram_tensor("vmix_out", [s, D], F32,
                              kind="ExternalOutput").ap()

    with ExitStack() as ctx:
        tc = ctx.enter_context(tile.TileContext(nc))

        # ---- persistent pools -------------------------------------------
        const_p = ctx.enter_context(tc.tile_pool(name="const", bufs=1))
        wo_pool = ctx.enter_context(tc.tile_pool(name="wo", bufs=1))
        x_pool = ctx.enter_context(tc.tile_pool(name="x", bufs=1))
        qk_pool = ctx.enter_context(tc.tile_pool(name="qk", bufs=1))

        ident = const_p.tile([P, P], F32, tag="ident")
        make_identity(nc, ident[:])
        identb = const_p.tile([P, P], BF16, tag="identb")
        make_identity(nc, identb[:])
        coup_sb = const_p.tile([P, 1], F32, tag="coup")
        nc.sync.dma_start(coup_sb[:], coup[:, :])

        wo_sb = wo_pool.tile([P, 2 * D], BF16, tag="wo")
        nc.sync.dma_start(
            wo_sb[:].rearrange("p (j c) -> p j c", c=D),
            wo_t.rearrange("(j p) c -> p j c", p=P))

        # X tiles: per s-chunk [128, XW] bf16, per head [v|sin|cos|one]
        x_tiles = [x_pool.tile([P, XW], BF16, tag=f"x{i}", name=f"x{i}")
                   for i in range(sc)]
        # qT/kT: [ch, s] bf16, 2 chunk-tiles each (heads 0,1 / heads 2,3)
        qT = [qk_pool.tile([P, s], BF16, tag=f"qT{c}", name=f"qT{c}")
              for c in range(2)]
        kT = [qk_pool.tile([P, s], BF16, tag=f"kT{c}", name=f"kT{c}")
              for c in range(2)]

        # ---- phase A/B: mag, transposes, QKV ----------------------------
        with tc.tile_pool(name="wqkv", bufs=1) as w_pool, \
             tc.tile_pool(name="zb", bufs=2) as z_pool, \
             tc.tile_pool(name="sq", bufs=2) as sq_pool, \
             tc.tile_pool(name="ssq", bufs=3) as ssq_pool, \
             tc.tile_pool(name="rv", bufs=2) as rv_pool, \
             tc.tile_pool(name="magt", bufs=DC + 1) as mag_pool, \
             tc.tile_pool(name="tp", bufs=3, space="PSUM") as tp_pool, \
             tc.tile_pool(name="pqk", bufs=3, space="PSUM") as pqk_pool, \
             tc.tile_pool(name="pv", bufs=2, space="PSUM") as pv_pool:

            # weights: one DMA each, chunked [128, .] by contraction rows
            wq_sb = w_pool.tile([P, DC * CH], BF16, tag="wq")
            wk_sb = w_pool.tile([P, DC * CH], BF16, tag="wk")
            wv_sb = w_pool.tile([P, DC * CH], BF16, tag="wv")
            for wsb, wdr in ((wq_sb, wq_t), (wk_sb, wk_t), (wv_sb, wv_t)):
                nc.sync.dma_start(
                    wsb[:].rearrange("p (j c) -> p j c", c=CH),
                    wdr.rearrange("(j p) c -> p j c", p=P))

            for h0, h1 in halves:
                nh = h1 - h0
                ssq_tiles = {}
                # -- elementwise + sin/cos per s-chunk
                for si in range(h0, h1):
                    zr = z_pool.tile([P, D], F32, tag="zr")
                    zi = z_pool.tile([P, D], F32, tag="zi")
                    nc.sync.dma_start(zr[:], z_re[si * P:(si + 1) * P, :])
                    nc.sync.dma_start(zi[:], z_im[si * P:(si + 1) * P, :])
                    t1 = sq_pool.tile([P, D], F32, tag="t1")
                    t2 = sq_pool.tile([P, D], F32, tag="t2")
                    nc.gpsimd.tensor_mul(t1[:], zr[:], zr[:])
                    nc.scalar.activation(t2[:], zi[:], AF.Square)
                    ssq = ssq_pool.tile([P, D], F32, tag="ssq")
                    nc.vector.tensor_add(ssq[:], t1[:], t2[:])
                    mag = ssq_pool.tile([P, D], BF16, tag="mag")
                    nc.scalar.activation(mag[:], ssq[:], AF.Sqrt)
                    ssq_tiles[si] = mag

                    # own channels are the first CH columns (host-permuted)
                    rinv = rv_pool.tile([P, CH], F32, tag="rinv")
                    nc.vector.reciprocal(rinv[:], mag[:, 0:CH])

                    x3 = x_tiles[si][:].rearrange("p (h c) -> p h c", c=256)
                    z3r = zr[:, 0:CH].rearrange("p (h c) -> p h c", c=HD)
                    z3i = zi[:, 0:CH].rearrange("p (h c) -> p h c", c=HD)
                    r3 = rinv[:].rearrange("p (h c) -> p h c", c=HD)
                    nc.vector.tensor_mul(x3[:, :, 64:128], z3i[:], r3[:])
                    nc.vector.tensor_mul(x3[:, :, 128:192], z3r[:], r3[:])
                    nc.vector.tensor_scalar(
                        out=x3[:, :, 192:193], in0=r3[:, :, 0:1],
                        scalar1=0.0, scalar2=1.0, op0=ALU.mult, op1=ALU.add)

                # -- transposes of mag (bf16) -> magT, DVE evict, groups of 2
                mag_t = [mag_pool.tile([P, nh * P], BF16, tag="magt",
                                       name=f"magt{j}")
                         for j in range(DC)]
                for g in range(0, nh, 2):
                    sa, sb_ = h0 + g, h0 + g + 1
                    for j in range(DC):
                        tp = tp_pool.tile([P, 256], BF16, tag="tp")
                        nc.tensor.transpose(
                            tp[:, 0:128], ssq_tiles[sa][:, j * P:(j + 1) * P],
                            identb[:])
                        if sb_ < h1:
                            nc.tensor.transpose(
                                tp[:, 128:256],
                                ssq_tiles[sb_][:, j * P:(j + 1) * P],
                                identb[:])
                        nc.vector.tensor_copy(
                            mag_t[j][:, g * P:(g + 2) * P], tp[:])

                # -- qT / kT for this half
                for c in range(2):
                    for t0 in range(0, nh * P, 512):
                        tw = min(512, nh * P - t0)
                        for wsb, dst in ((wq_sb, qT[c]), (wk_sb, kT[c])):
                            ps = pqk_pool.tile([P, 512], F32, tag="pqk")
                            for j in range(DC):
                                nc.tensor.matmul(
                                    ps[:, 0:tw],
                                    wsb[:, j * CH + c * P:j * CH + (c + 1) * P],
                                    mag_t[j][:, t0:t0 + tw],
                                    start=(j == 0), stop=(j == DC - 1))
                            nc.vector.tensor_copy(
                                dst[:, h0 * P + t0:h0 * P + t0 + tw],
                                ps[:, 0:tw])

                # -- v for this half -> X v cols
                for ii in range(nh):
                    si = h0 + ii
                    psv = pv_pool.tile([P, CH], F32, tag="pv")
                    for j in range(DC):
                        nc.tensor.matmul(
                            psv[:], mag_t[j][:, ii * P:(ii + 1) * P],
                            wv_sb[:, j * CH:(j + 1) * CH],
                            start=(j == 0), stop=(j == DC - 1))
                    x3 = x_tiles[si][:].rearrange("p (h c) -> p h c", c=256)
                    p3 = psv[:].rearrange("p (h c) -> p h c", c=HD)
                    nc.vector.tensor_copy(x3[:, :, 0:HD], p3[:])

        # ---- phase C/D: pipelined attention + output --------------------
        units = [(qt, h) for qt in range(n_qt) for h in range(HPC)]
        n_units = len(units)

        with tc.tile_pool(name="wt", bufs=2 * kc) as wt_pool, \
             tc.tile_pool(name="vout", bufs=2 * uq) as vout_pool, \
             tc.tile_pool(name="vT", bufs=2) as vT_pool, \
             tc.tile_pool(name="ph", bufs=2 * uq) as ph_pool, \
             tc.tile_pool(name="vm", bufs=2) as vm_pool, \
             tc.tile_pool(name="rc", bufs=8) as rc_pool, \
             tc.tile_pool(name="tmp", bufs=4) as tmp_pool, \
             tc.tile_pool(name="pss", bufs=2, space="PSUM") as pss_pool, \
             tc.tile_pool(name="pso", bufs=2, space="PSUM") as pso_pool, \
             tc.tile_pool(name="pd", bufs=1, space="PSUM") as pd_pool:

            wt_sets = [None] * n_units
            vout_sets = {}
            ph_sets = {}

            def issue_scores(su, k, wt):
                qt, h = units[su]
                c, hh = divmod(h, 2)
                q0 = (qt * uq) * P
                ps = pss_pool.tile([P, qw], F32, tag="pss", name="pss")
                for qh in range(0, qw, 512):
                    qn = min(512, qw - qh)
                    nc.tensor.matmul(
                        ps[:, qh:qh + qn],
                        kT[c][hh * 64:(hh + 1) * 64, k * P:(k + 1) * P],
                        qT[c][hh * 64:(hh + 1) * 64, q0 + qh:q0 + qh + qn],
                        start=True, stop=True)
                nc.scalar.activation(wt[:], ps[:], AF.Exp, scale=0.125)

            def issue_mix(pu, u, ps, off):
                qt, h = units[pu]
                wts = wt_sets[pu]
                for k in range(kc):
                    nc.tensor.matmul(
                        ps[:, off:off + XN],
                        wts[k][:, u * P:(u + 1) * P],
                        x_tiles[k][:, h * 256:h * 256 + XN],
                        start=(k == 0), stop=(k == kc - 1))

            def issue_epilogue(pu, u, ps, off):
                qt, h = units[pu]
                si = qt * uq + u
                rec = rc_pool.tile([P, 1], F32, tag="rec", name="rec")
                rcp = rc_pool.tile([P, 1], F32, tag="rcp", name="rcp")
                nc.vector.reciprocal(rec[:], ps[:, off + 192:off + 193])
                nc.vector.tensor_scalar_mul(rcp[:], rec[:], coup_sb[:])
                nc.vector.tensor_scalar(
                    out=vout_sets[qt][u][:, h * HD:(h + 1) * HD],
                    in0=ps[:, off:off + HD], scalar1=rec[:], scalar2=None,
                    op0=ALU.mult)
                x3 = x_tiles[si][:].rearrange("p (h c) -> p h c", c=256)
                cos_src = x3[:, h, 128:192]
                sin_src = x3[:, h, 64:128]
                ta = tmp_pool.tile([P, HD], F32, tag="ta", name="ta")
                tb = tmp_pool.tile([P, HD], F32, tag="tb", name="tb")
                nc.vector.scalar_tensor_tensor(
                    out=ta[:], in0=ps[:, off + 64:off + 128], scalar=rcp[:],
                    in1=cos_src, op0=ALU.mult, op1=ALU.mult)
                nc.vector.scalar_tensor_tensor(
                    out=tb[:], in0=ps[:, off + 128:off + 192], scalar=rcp[:],
                    in1=sin_src, op0=ALU.mult, op1=ALU.mult)
                nc.vector.tensor_sub(
                    ph_sets[qt][u][:, h * HD:(h + 1) * HD], ta[:], tb[:])
                if h == HPC - 1:
                    nc.sync.dma_start(
                        phase_out[si * P:(si + 1) * P, :], ph_sets[qt][u][:])

            def issue_phase_d(qt):
                t0 = qt * uq
                vT = [vT_pool.tile([P, qw], BF16, tag=f"vT{c}",
                                   name=f"vT{c}") for c in range(2)]
                for c in range(2):
                    for g in range(0, uq, 4):
                        gn = min(4, uq - g)
                        tp = pd_pool.tile([P, 512], BF16, tag="pdT",
                                          name="pdT", bufs=1)
                        for u in range(g, g + gn):
                            nc.tensor.transpose(
                                tp[:, (u - g) * P:(u - g + 1) * P],
                                vout_sets[qt][u][:, c * P:(c + 1) * P],
                                identb[:])
                        nc.vector.tensor_copy(
                            vT[c][:, g * P:(g + gn) * P], tp[:, 0:gn * P])
                for u in range(uq):
                    su_ = t0 + u
                    vm = vm_pool.tile([P, D], F32, tag="vm", name="vm")
                    for iti, it in enumerate(range(0, D, 512)):
                        psm = pd_pool.tile([P, 512], F32, tag="pdM",
                                           name="pdM")
                        for c in range(2):
                            nc.tensor.matmul(
                                psm[:],
                                vT[c][:, u * P:(u + 1) * P],
                                wo_sb[:, c * D + it:c * D + it + 512],
                                start=(c == 0), stop=(c == 1))
                        # split evictions DVE/ACT to balance engine load
                        if iti % 2 == 0:
                            nc.vector.tensor_copy(
                                vm[:, it:it + 512], psm[:])
                        else:
                            nc.scalar.activation(
                                vm[:, it:it + 512], psm[:], AF.Copy)
                    nc.sync.dma_start(
                        vmix_out[su_ * P:(su_ + 1) * P, :], vm[:])

            for s2 in range(n_units + 1):
                cur = s2 if s2 < n_units else None
                prev = s2 - 1 if s2 > 0 else None
                if cur is not None:
                    wt_sets[cur] = [
                        wt_pool.tile([P, qw], BF16, tag="wt",
                                     name=f"wt{cur}_{k}")
                        for k in range(kc)]
                if prev is not None and units[prev][1] == 0:
                    pqt = units[prev][0]
                    vout_sets[pqt] = [
                        vout_pool.tile([P, CH], BF16, tag="vout",
                                       name=f"vo{pqt}_{u}")
                        for u in range(uq)]
                    ph_sets[pqt] = [
                        ph_pool.tile([P, CH], F32, tag="ph",
                                     name=f"ph{pqt}_{u}")
                        for u in range(uq)]
                psos = {}
                for u in range(uq):
                    if cur is not None:
                        for k in range(u * kpu, (u + 1) * kpu):
                            issue_scores(cur, k, wt_sets[cur][k])
                    if prev is not None:
                        j, off = u // 2, (u % 2) * 256
                        if u % 2 == 0:
                            psos[j] = pso_pool.tile([P, 512], F32,
                                                    tag="pso", name="pso")
                        issue_mix(prev, u, psos[j], off)
                        issue_epilogue(prev, u, psos[j], off)
                if prev is not None and units[prev][1] == HPC - 1:
                    issue_phase_d(units[prev][0])

    if not nc.is_finalized():
        nc.finalize()
    return nc


_CACHED = {}


def _get_nc(sc=SC):
    if sc not in _CACHED:
        _CACHED[sc] = build_kernel(sc)
    return _CACHED[sc]


def make_in_maps(z_re, z_im, Wq, Wk, Wv, Wo, coupling_strength, sc=SC):
    """Host-side sharding: core = b * HG + hg.  z columns (and W{q,k,v}
    contraction rows) are permuted so the core's own channels come first."""
    z_re = np.asarray(z_re, dtype=np.float32)
    z_im = np.asarray(z_im, dtype=np.float32)
    Wq = np.asarray(Wq, dtype=np.float32)
    Wk = np.asarray(Wk, dtype=np.float32)
    Wv = np.asarray(Wv, dtype=np.float32)
    Wo = np.asarray(Wo, dtype=np.float32)
    coup_b = np.full((P, 1), np.float32(np.asarray(coupling_strength)),
                     dtype=np.float32)
    s = sc * P
    bf = ml_dtypes.bfloat16
    in_maps = []
    for core in range(N_CORES):
        b, hg = divmod(core, HG)
        lo, hi = hg * CH, (hg + 1) * CH
        perm = np.r_[lo:hi, 0:lo, hi:D]
        in_maps.append({
            "z_re": np.ascontiguousarray(z_re[b, :s][:, perm]),
            "z_im": np.ascontiguousarray(z_im[b, :s][:, perm]),
            "wq_t": np.ascontiguousarray(Wq[lo:hi][:, perm].T.astype(bf)),
            "wk_t": np.ascontiguousarray(Wk[lo:hi][:, perm].T.astype(bf)),
            "wv_t": np.ascontiguousarray(Wv[lo:hi][:, perm].T.astype(bf)),
            "wo_t": np.ascontiguousarray(Wo[:, lo:hi].T.astype(bf)),
            "coup": coup_b,
        })
    return in_maps


def gather_outputs(results, sc=SC):
    s = sc * P
    phase = np.zeros((B, s, D), dtype=np.float32)
    vmix = np.zeros((B, s, D), dtype=np.float32)
    for core in range(N_CORES):
        b, hg = divmod(core, HG)
        lo, hi = hg * CH, (hg + 1) * CH
        phase[b, :, lo:hi] = results[core]["phase_out"]
        vmix[b] += results[core]["vmix_out"]
    return phase, vmix


def kernel(z_re, z_im, Wq, Wk, Wv, Wo, coupling_strength):
    nc = _get_nc(SC)
    in_maps = make_in_maps(z_re, z_im, Wq, Wk, Wv, Wo, coupling_strength)
    res = run_bass_kernel_spmd(nc, in_maps, core_ids=list(range(N_CORES)))
    return gather_outputs(res.results)


# revision 18
# speedup vs baseline: 1.9809x; 1.9809x over previous
"""Trainium2 Bass kernel for nn_KuramotoCoupling.

Reference computation (B=2, S=2048, D=1024, H=16, HD=64):
    mag   = sqrt(z_re^2 + z_im^2)
    q,k,v = mag @ W{q,k,v}.T  (per-head, HD=64)
    w     = softmax(q k^T / 8)
    value_mix    = (w @ v) @ Wo.T
    phase_update = c * (cos_ph * (w @ sin_ph) - sin_ph * (w @ cos_ph))
where sin_ph = z_im / mag, cos_ph = z_re / mag  (since ph = atan2(z_im, z_re)).

Sharding: 8 cores = 2 batches x 4 head-groups (4 heads each).  Each core
gets its batch's z (full D, column-permuted so its OWN 256 channels come
first), its heads' slices of Wq/Wk/Wv (rows permuted to match, bf16), and
its j-slice of Wo^T (bf16).  Outputs: phase slice [S, 256] (concatenated
on host) and a value_mix partial [S, D] (summed over head-groups on host).

Device-side structure per core:
  A) stream z by 128-row chunks: ssq = re^2+im^2 (gpsimd square + ACT
     square + DVE add); own channels = ssq[:, 0:256] -> rinv -> sin/cos
     into X tiles [s, (head: v|sin|cos|one)] (bf16); PE-transpose ssq and
     evict through ACT Sqrt -> magT [d, s] (bf16).
  B) qT/kT = WqT.T @ magT  [ch, s] (bf16), v = magT.T @ WvT -> X v cols.
  C) flat software pipeline over 8 units (2 q-tiles x 4 heads): for unit
     N+1 the scoresT = kT.T @ qT -> ACT exp (bf16 wT, no max-sub:
     |scores/8| < ~6) interleaved at k-granularity with unit N's mixing
     out[q, v|sin|cos|one] += wT.T @ X (accumulated over all 16 k in
     PSUM).  The one-column gives the softmax denominator.  Per-u DVE
     epilogue: recip, value rows scaled, phase = (Ksin*rc)*cos -
     (Kcos*rc)*sin.  PE and ACT overlap across units.
  D) per q-tile: PE-transpose value rows (bf16) -> vT, value_mix
     partial [q, i] = vT.T @ WoT, DMA'd straight from PSUM.
"""
import sys

if "/opt/trn_rl_repo" not in sys.path:
    sys.path.insert(0, "/opt/trn_rl_repo")

import numpy as np
import ml_dtypes
from contextlib import ExitStack

import concourse.bacc as bacc
import concourse.mybir as mybir
import concourse.tile as tile
from concourse.bass_utils import run_bass_kernel_spmd
from concourse.masks import make_identity

F32 = mybir.dt.float32
F32R = mybir.dt.float32r
BF16 = mybir.dt.bfloat16
AF = mybir.ActivationFunctionType
ALU = mybir.AluOpType

B, S, D, H = 2, 2048, 1024, 16
HD = D // H
N_CORES = 8
HG = 4                   # head-groups (tensor-parallel dim)
HPC = H // HG            # heads per core = 4
CH = HPC * HD            # own channels per core = 256
P = 128
SC = S // P              # 16 s-chunks
DC = D // P              # 8 d-chunks
XW = HPC * 256           # X tile width: per head [v64|sin64|cos64|one..]
XN = 193                 # mixing moving width: v64|sin64|cos64|one


def build_kernel(sc=SC):
    """Build the SPMD program for one core.  sc = number of 128-row
    s-chunks (16 for the real problem; smaller for debug builds)."""
    s = sc * P
    kc = sc                      # k-chunks == s-chunks
    uq = max(1, sc // 2)         # q-chunks per q-tile
    n_qt = 2 if sc >= 2 else 1
    qw = uq * P
    kpu = kc // uq               # score k-chunks issued per u-step (2)
    gq = 4 if sc % 4 == 0 else sc    # s-chunks per phase-A/B group
    groups = [(g, g + gq) for g in range(0, sc, gq)]

    nc = bacc.Bacc("TRN2", target_bir_lowering=False, debug=False,
                   num_devices=N_CORES)

    z_re = nc.dram_tensor("z_re", [s, D], F32, kind="ExternalInput").ap()
    z_im = nc.dram_tensor("z_im", [s, D], F32, kind="ExternalInput").ap()
    wq_t = nc.dram_tensor("wq_t", [D, CH], BF16, kind="ExternalInput").ap()
    wk_t = nc.dram_tensor("wk_t", [D, CH], BF16, kind="ExternalInput").ap()
    wv_t = nc.dram_tensor("wv_t", [D, CH], BF16, kind="ExternalInput").ap()
    wo_t = nc.dram_tensor("wo_t", [CH, D], BF16, kind="ExternalInput").ap()
    coup = nc.dram_tensor("coup", [P, 1], F32, kind="ExternalInput").ap()
    phase_out = nc.dram_tensor("phase_out", [s, CH], F32,
                               kind="ExternalOutput").ap()
    vmix_out = nc.dram_tensor("vmix_out", [s, D], F32,
                              kind="ExternalOutput").ap()

    with ExitStack() as ctx:
        tc = ctx.enter_context(tile.TileContext(nc))

        # ---- persistent pools -------------------------------------------
        const_p = ctx.enter_context(tc.tile_pool(name="const", bufs=1))
        wo_pool = ctx.enter_context(tc.tile_pool(name="wo", bufs=1))
        x_pool = ctx.enter_context(tc.tile_pool(name="x", bufs=1))
        qk_pool = ctx.enter_context(tc.tile_pool(name="qk", bufs=1))

        ident = const_p.tile([P, P], F32, tag="ident")
        make_identity(nc, ident[:])
        identb = const_p.tile([P, P], BF16, tag="identb")
        make_identity(nc, identb[:])
        coup_sb = const_p.tile([P, 1], F32, tag="coup")
        nc.sync.dma_start(coup_sb[:], coup[:, :])

        wo_sb = wo_pool.tile([P, 2 * D], BF16, tag="wo")
        nc.sync.dma_start(
            wo_sb[:].rearrange("p (j c) -> p j c", c=D),
            wo_t.rearrange("(j p) c -> p j c", p=P))

        # X tiles: per s-chunk [128, XW] bf16, per head [v|sin|cos|one]
        x_tiles = [x_pool.tile([P, XW], BF16, tag=f"x{i}", name=f"x{i}")
                   for i in range(sc)]
        # qT/kT: [ch, s] bf16, 2 chunk-tiles each (heads 0,1 / heads 2,3)
        qT = [qk_pool.tile([P, s], BF16, tag=f"qT{c}", name=f"qT{c}")
              for c in range(2)]
        kT = [qk_pool.tile([P, s], BF16, tag=f"kT{c}", name=f"kT{c}")
              for c in range(2)]

        # ---- phase A/B: mag, transposes, QKV ----------------------------
        with tc.tile_pool(name="wqkv", bufs=1) as w_pool, \
             tc.tile_pool(name="zb", bufs=3) as z_pool, \
             tc.tile_pool(name="sq", bufs=2) as sq_pool, \
             tc.tile_pool(name="ssq", bufs=3) as ssq_pool, \
             tc.tile_pool(name="rv", bufs=2) as rv_pool, \
             tc.tile_pool(name="magt", bufs=DC + 1) as mag_pool, \
             tc.tile_pool(name="tp", bufs=3, space="PSUM") as tp_pool, \
             tc.tile_pool(name="pqk", bufs=3, space="PSUM") as pqk_pool, \
             tc.tile_pool(name="pv", bufs=2, space="PSUM") as pv_pool:

            # weights: one DMA each, chunked [128, .] by contraction rows
            wq_sb = w_pool.tile([P, DC * CH], BF16, tag="wq")
            wk_sb = w_pool.tile([P, DC * CH], BF16, tag="wk")
            wv_sb = w_pool.tile([P, DC * CH], BF16, tag="wv")
            for wsb, wdr in ((wq_sb, wq_t), (wk_sb, wk_t), (wv_sb, wv_t)):
                nc.sync.dma_start(
                    wsb[:].rearrange("p (j c) -> p j c", c=CH),
                    wdr.rearrange("(j p) c -> p j c", p=P))

            for h0, h1 in groups:
                nh = h1 - h0
                ssq_tiles = {}
                # -- elementwise + sin/cos per s-chunk
                for si in range(h0, h1):
                    zr = z_pool.tile([P, D], F32, tag="zr")
                    zi = z_pool.tile([P, D], F32, tag="zi")
                    nc.sync.dma_start(zr[:], z_re[si * P:(si + 1) * P, :])
                    nc.sync.dma_start(zi[:], z_im[si * P:(si + 1) * P, :])
                    t1 = sq_pool.tile([P, D], F32, tag="t1")
                    t2 = sq_pool.tile([P, D], F32, tag="t2")
                    nc.gpsimd.tensor_mul(t1[:], zr[:], zr[:])
                    nc.scalar.activation(t2[:], zi[:], AF.Square)
                    ssq = ssq_pool.tile([P, D], F32, tag="ssq")
                    nc.vector.tensor_add(ssq[:], t1[:], t2[:])
                    mag = ssq_pool.tile([P, D], BF16, tag="mag")
                    nc.scalar.activation(mag[:], ssq[:], AF.Sqrt)
                    ssq_tiles[si] = mag

                    # own channels are the first CH columns (host-permuted)
                    rinv = rv_pool.tile([P, CH], F32, tag="rinv")
                    nc.vector.reciprocal(rinv[:], mag[:, 0:CH])

                    x3 = x_tiles[si][:].rearrange("p (h c) -> p h c", c=256)
                    z3r = zr[:, 0:CH].rearrange("p (h c) -> p h c", c=HD)
                    z3i = zi[:, 0:CH].rearrange("p (h c) -> p h c", c=HD)
                    r3 = rinv[:].rearrange("p (h c) -> p h c", c=HD)
                    nc.vector.tensor_mul(x3[:, :, 64:128], z3i[:], r3[:])
                    nc.vector.tensor_mul(x3[:, :, 128:192], z3r[:], r3[:])
                    nc.vector.tensor_scalar(
                        out=x3[:, :, 192:193], in0=r3[:, :, 0:1],
                        scalar1=0.0, scalar2=1.0, op0=ALU.mult, op1=ALU.add)

                # -- transposes of mag (bf16) -> magT, DVE evict, groups of 2
                mag_t = [mag_pool.tile([P, nh * P], BF16, tag="magt",
                                       name=f"magt{j}")
                         for j in range(DC)]
                for g in range(0, nh, 2):
                    sa, sb_ = h0 + g, h0 + g + 1
                    for j in range(DC):
                        tp = tp_pool.tile([P, 256], BF16, tag="tp")
                        nc.tensor.transpose(
                            tp[:, 0:128], ssq_tiles[sa][:, j * P:(j + 1) * P],
                            identb[:])
                        if sb_ < h1:
                            nc.tensor.transpose(
                                tp[:, 128:256],
                                ssq_tiles[sb_][:, j * P:(j + 1) * P],
                                identb[:])
                        nc.vector.tensor_copy(
                            mag_t[j][:, g * P:(g + 2) * P], tp[:])

                # -- qT / kT for this half
                for c in range(2):
                    for t0 in range(0, nh * P, 512):
                        tw = min(512, nh * P - t0)
                        for wsb, dst in ((wq_sb, qT[c]), (wk_sb, kT[c])):
                            ps = pqk_pool.tile([P, 512], F32, tag="pqk")
                            for j in range(DC):
                                nc.tensor.matmul(
                                    ps[:, 0:tw],
                                    wsb[:, j * CH + c * P:j * CH + (c + 1) * P],
                                    mag_t[j][:, t0:t0 + tw],
                                    start=(j == 0), stop=(j == DC - 1))
                            nc.vector.tensor_copy(
                                dst[:, h0 * P + t0:h0 * P + t0 + tw],
                                ps[:, 0:tw])

                # -- v for this half -> X v cols
                for ii in range(nh):
                    si = h0 + ii
                    psv = pv_pool.tile([P, CH], F32, tag="pv")
                    for j in range(DC):
                        nc.tensor.matmul(
                            psv[:], mag_t[j][:, ii * P:(ii + 1) * P],
                            wv_sb[:, j * CH:(j + 1) * CH],
                            start=(j == 0), stop=(j == DC - 1))
                    x3 = x_tiles[si][:].rearrange("p (h c) -> p h c", c=256)
                    p3 = psv[:].rearrange("p (h c) -> p h c", c=HD)
                    nc.vector.tensor_copy(x3[:, :, 0:HD], p3[:])

        # ---- phase C/D: pipelined attention + output --------------------
        units = [(qt, h) for qt in range(n_qt) for h in range(HPC)]
        n_units = len(units)

        with tc.tile_pool(name="wt", bufs=2 * kc) as wt_pool, \
             tc.tile_pool(name="vout", bufs=2 * uq) as vout_pool, \
             tc.tile_pool(name="vT", bufs=2) as vT_pool, \
             tc.tile_pool(name="ph", bufs=2 * uq) as ph_pool, \
             tc.tile_pool(name="vm", bufs=2) as vm_pool, \
             tc.tile_pool(name="rc", bufs=8) as rc_pool, \
             tc.tile_pool(name="tmp", bufs=4) as tmp_pool, \
             tc.tile_pool(name="pss", bufs=2, space="PSUM") as pss_pool, \
             tc.tile_pool(name="pso", bufs=3, space="PSUM") as pso_pool, \
             tc.tile_pool(name="pd", bufs=1, space="PSUM") as pd_pool:

            wt_sets = [None] * n_units
            vout_sets = {}
            ph_sets = {}

            def issue_scores(su, k, wt):
                qt, h = units[su]
                c, hh = divmod(h, 2)
                q0 = (qt * uq) * P
                ps = pss_pool.tile([P, qw], F32, tag="pss", name="pss")
                for qh in range(0, qw, 512):
                    qn = min(512, qw - qh)
                    nc.tensor.matmul(
                        ps[:, qh:qh + qn],
                        kT[c][hh * 64:(hh + 1) * 64, k * P:(k + 1) * P],
                        qT[c][hh * 64:(hh + 1) * 64, q0 + qh:q0 + qh + qn],
                        start=True, stop=True)
                nc.scalar.activation(wt[:], ps[:], AF.Exp, scale=0.125)

            def issue_mix(pu, u, ps, off):
                qt, h = units[pu]
                wts = wt_sets[pu]
                for k in range(kc):
                    nc.tensor.matmul(
                        ps[:, off:off + XN],
                        wts[k][:, u * P:(u + 1) * P],
                        x_tiles[k][:, h * 256:h * 256 + XN],
                        start=(k == 0), stop=(k == kc - 1))

            def issue_recips(ps, nu):
                """Batched softmax denominators for the nu (1 or 2) u's
                sharing psum tile ps: ones cols at 192, 448."""
                rec = rc_pool.tile([P, 2], F32, tag="rec", name="rec")
                rcp = rc_pool.tile([P, 2], F32, tag="rcp", name="rcp")
                ps3 = ps[:].rearrange("p (j c) -> p j c", c=256)
                nc.vector.reciprocal(
                    rec[:, 0:nu].rearrange("p (j c) -> p j c", c=1),
                    ps3[:, 0:nu, 192:193])
                nc.vector.tensor_scalar_mul(rcp[:, 0:nu], rec[:, 0:nu],
                                            coup_sb[:])
                return rec, rcp

            def issue_epilogue(pu, u, ps, off, recs):
                qt, h = units[pu]
                si = qt * uq + u
                rec = recs[0][:, u % 2:u % 2 + 1]
                rcp = recs[1][:, u % 2:u % 2 + 1]
                nc.vector.tensor_scalar(
                    out=vout_sets[qt][u][:, h * HD:(h + 1) * HD],
                    in0=ps[:, off:off + HD], scalar1=rec, scalar2=None,
                    op0=ALU.mult)
                x3 = x_tiles[si][:].rearrange("p (h c) -> p h c", c=256)
                cos_src = x3[:, h, 128:192]
                sin_src = x3[:, h, 64:128]
                ta = tmp_pool.tile([P, HD], F32, tag="ta", name="ta")
                tb = tmp_pool.tile([P, HD], F32, tag="tb", name="tb")
                nc.vector.scalar_tensor_tensor(
                    out=ta[:], in0=ps[:, off + 64:off + 128], scalar=rcp,
                    in1=cos_src, op0=ALU.mult, op1=ALU.mult)
                nc.vector.scalar_tensor_tensor(
                    out=tb[:], in0=ps[:, off + 128:off + 192], scalar=rcp,
                    in1=sin_src, op0=ALU.mult, op1=ALU.mult)
                nc.vector.tensor_sub(
                    ph_sets[qt][u][:, h * HD:(h + 1) * HD], ta[:], tb[:])
                if h == HPC - 1:
                    nc.sync.dma_start(
                        phase_out[si * P:(si + 1) * P, :], ph_sets[qt][u][:])

            def issue_phase_d(qt):
                t0 = qt * uq
                vT = [vT_pool.tile([P, qw], BF16, tag=f"vT{c}",
                                   name=f"vT{c}") for c in range(2)]
                for c in range(2):
                    for g in range(0, uq, 4):
                        gn = min(4, uq - g)
                        tp = pd_pool.tile([P, 512], BF16, tag="pdT",
                                          name="pdT", bufs=1)
                        for u in range(g, g + gn):
                            nc.tensor.transpose(
                                tp[:, (u - g) * P:(u - g + 1) * P],
                                vout_sets[qt][u][:, c * P:(c + 1) * P],
                                identb[:])
                        nc.vector.tensor_copy(
                            vT[c][:, g * P:(g + gn) * P], tp[:, 0:gn * P])
                for u in range(uq):
                    su_ = t0 + u
                    vm = vm_pool.tile([P, D], F32, tag="vm", name="vm")
                    for iti, it in enumerate(range(0, D, 512)):
                        psm = pso_pool.tile([P, 512], F32, tag="pso",
                                            name="pdM")
                        for c in range(2):
                            nc.tensor.matmul(
                                psm[:],
                                vT[c][:, u * P:(u + 1) * P],
                                wo_sb[:, c * D + it:c * D + it + 512],
                                start=(c == 0), stop=(c == 1))
                        # split evictions DVE/ACT to balance engine load
                        if iti % 2 == 0:
                            nc.vector.tensor_copy(
                                vm[:, it:it + 512], psm[:])
                        else:
                            nc.scalar.activation(
                                vm[:, it:it + 512], psm[:], AF.Copy)
                    nc.sync.dma_start(
                        vmix_out[su_ * P:(su_ + 1) * P, :], vm[:])

            for s2 in range(n_units + 1):
                cur = s2 if s2 < n_units else None
                prev = s2 - 1 if s2 > 0 else None
                if cur is not None:
                    wt_sets[cur] = [
                        wt_pool.tile([P, qw], BF16, tag="wt",
                                     name=f"wt{cur}_{k}")
                        for k in range(kc)]
                if prev is not None and units[prev][1] == 0:
                    pqt = units[prev][0]
                    vout_sets[pqt] = [
                        vout_pool.tile([P, CH], BF16, tag="vout",
                                       name=f"vo{pqt}_{u}")
                        for u in range(uq)]
                    ph_sets[pqt] = [
                        ph_pool.tile([P, CH], F32, tag="ph",
                                     name=f"ph{pqt}_{u}")
                        for u in range(uq)]
                psos = {}
                for u in range(uq):
                    if cur is not None:
                        for k in range(u * kpu, (u + 1) * kpu):
                            issue_scores(cur, k, wt_sets[cur][k])
                    if prev is not None:
                        j, off = u // 2, (u % 2) * 256
                        if u % 2 == 0:
                            psos[j] = pso_pool.tile([P, 512], F32,
                                                    tag="pso", name="pso")
                        issue_mix(prev, u, psos[j], off)
                        if u % 2 == 1 or u == uq - 1:
                            recs = issue_recips(psos[j], u % 2 + 1)
                            for uu in range(j * 2, u + 1):
                                issue_epilogue(prev, uu, psos[j],
                                               (uu % 2) * 256, recs)
                if prev is not None and units[prev][1] == HPC - 1:
                    issue_phase_d(units[prev][0])

    if not nc.is_finalized():
        nc.finalize()
    return nc


_CACHED = {}


def _get_nc(sc=SC):
    if sc not in _CACHED:
        _CACHED[sc] = build_kernel(sc)
    return _CACHED[sc]


def make_in_maps(z_re, z_im, Wq, Wk, Wv, Wo, coupling_strength, sc=SC):
    """Host-side sharding: core = b * HG + hg.  z columns (and W{q,k,v}
    contraction rows) are permuted so the core's own channels come first."""
    z_re = np.asarray(z_re, dtype=np.float32)
    z_im = np.asarray(z_im, dtype=np.float32)
    Wq = np.asarray(Wq, dtype=np.float32)
    Wk = np.asarray(Wk, dtype=np.float32)
    Wv = np.asarray(Wv, dtype=np.float32)
    Wo = np.asarray(Wo, dtype=np.float32)
    coup_b = np.full((P, 1), np.float32(np.asarray(coupling_strength)),
                     dtype=np.float32)
    s = sc * P
    bf = ml_dtypes.bfloat16
    in_maps = []
    for core in range(N_CORES):
        b, hg = divmod(core, HG)
        lo, hi = hg * CH, (hg + 1) * CH
        perm = np.r_[lo:hi, 0:lo, hi:D]
        in_maps.append({
            "z_re": np.ascontiguousarray(z_re[b, :s][:, perm]),
            "z_im": np.ascontiguousarray(z_im[b, :s][:, perm]),
            "wq_t": np.ascontiguousarray(Wq[lo:hi][:, perm].T.astype(bf)),
            "wk_t": np.ascontiguousarray(Wk[lo:hi][:, perm].T.astype(bf)),
            "wv_t": np.ascontiguousarray(Wv[lo:hi][:, perm].T.astype(bf)),
            "wo_t": np.ascontiguousarray(Wo[:, lo:hi].T.astype(bf)),
            "coup": coup_b,
        })
    return in_maps


def gather_outputs(results, sc=SC):
    s = sc * P
    phase = np.zeros((B, s, D), dtype=np.float32)
    vmix = np.zeros((B, s, D), dtype=np.float32)
    for core in range(N_CORES):
        b, hg = divmod(core, HG)
        lo, hi = hg * CH, (hg + 1) * CH
        phase[b, :, lo:hi] = results[core]["phase_out"]
        vmix[b] += results[core]["vmix_out"]
    return phase, vmix


def kernel(z_re, z_im, Wq, Wk, Wv, Wo, coupling_strength):
    nc = _get_nc(SC)
    in_maps = make_in_maps(z_re, z_im, Wq, Wk, Wv, Wo, coupling_strength)
    res = run_bass_kernel_spmd(nc, in_maps, core_ids=list(range(N_CORES)))
    return gather_outputs(res.results)
